# revision 17
# baseline (speedup 1.0000x reference)
"""Trainium2 Bass kernel for nn_ContactForceModel: 2-layer LSTM (B=512,T=128,D=64,H=512)
+ per-sphere decoder MLP. Data-parallel over batch across 4 NeuronCores (128 rows/core)
so every matmul fills the full 128-wide PE array (M=Bc=128).

v2 strategy (per core, Bc=128):
- All matmul operands bf16 (1 cyc/row at any N; PSUM accumulates fp32). Host
  converts weights/x once; no on-device f32r staging.
- LSTM matmuls stream weights (N=512 slices) against stationary activation
  k-tiles (lhsT = h^T [128, 128]); k-outer/n-inner order reuses each stationary.
- h^T rebuilt each step by ONE XBAR dma_start_transpose [128,512]bf16 ->
  [128,4,128] (verified: hT[p,k,b] = h[b, 128k+p], matching the ktile weight
  layout) instead of DVE block-transpose + 8 DMAs.
- Layer 1 lags layer 0 by one step so both layers' matmuls keep the PE dense;
  elementwise (Act/DVE) hides under the other layer's matmul stream.
- Decoder exploits rank-1 feature structure: feat @ W0 = sphere_proj (host)
  (+) latent_proj (device) broadcast-add; softplus(x) = relu(x) + ln(exp(-|x|)+1).
"""

import sys

sys.path.insert(0, "/opt/trn_rl_repo")

import ml_dtypes
import numpy as np

import concourse.bass as bass
import concourse.tile as tile
from concourse import bacc, mybir
from concourse.bass_utils import run_bass_kernel_spmd

F32 = mybir.dt.float32
BF16 = mybir.dt.bfloat16
FP8 = mybir.dt.float8e4
AF = mybir.ActivationFunctionType
BF = ml_dtypes.bfloat16
F8 = ml_dtypes.float8_e4m3

N_CORES = 4
HID = 512
NK = HID // 128  # 4 k-tiles over the hidden dim

# fp8e4m3 DoubleRow matmuls for the three recurrent weight matrices (2x PE
# throughput). Weights are pre-scaled by W_SCALE on host (avoids fp8
# subnormals); gate activations apply 1/W_SCALE.
USE_FP8 = True
W_SCALE = 64.0


def build_module(T, Bc, K0, S, use_bl1, use_b1, bout_f):
    G = 4 * HID  # 2048
    nc = bacc.Bacc("TRN2", target_bir_lowering=False, debug=False)

    WDT = FP8 if USE_FP8 else BF16
    xT_ext = nc.declare_dram_parameter("xT", [K0, T * Bc], BF16, isOutput=False)
    wx0_ext = nc.declare_dram_parameter("wx0", [K0, G], BF16, isOutput=False)
    wh0_ext = nc.declare_dram_parameter("wh0", [128, NK * G], WDT, isOutput=False)
    wx1_ext = nc.declare_dram_parameter("wx1", [128, NK * G], WDT, isOutput=False)
    wh1_ext = nc.declare_dram_parameter("wh1", [128, NK * G], WDT, isOutput=False)
    spT_ext = nc.declare_dram_parameter("spT", [128, S], F32, isOutput=False)
    w0h_ext = nc.declare_dram_parameter("w0h", [128, NK * 128], BF16, isOutput=False)
    w1_ext = nc.declare_dram_parameter("w1", [128, 64], BF16, isOutput=False)
    wout_ext = nc.declare_dram_parameter("wout", [64, 1], BF16, isOutput=False)
    bl1_ext = nc.declare_dram_parameter("bl1", [1, G], BF16, isOutput=False) if use_bl1 else None
    b1_ext = nc.declare_dram_parameter("b1", [1, 64], BF16, isOutput=False) if use_b1 else None
    out_ext = nc.declare_dram_parameter("out", [Bc, S], F32, isOutput=True)

    R = Bc * S  # decoder rows per core
    NCH = R // 2048  # 2048-row chunks

    with tile.TileContext(nc) as tc:
        with tc.tile_pool(name="dwts", bufs=1) as dw, \
             tc.tile_pool(name="state", bufs=1) as st:

            # ---------- persistent state ----------
            c0 = st.tile([Bc, HID], F32)
            c1 = st.tile([Bc, HID], F32)
            nc.vector.memset(c0, 0.0)
            nc.vector.memset(c1, 0.0)
            h0T = st.tile([128, NK, Bc], BF16)
            h1T = st.tile([128, NK, Bc], BF16)
            nc.vector.memset(h0T, 0.0)
            nc.vector.memset(h1T, 0.0)
            h0T8 = h1T8 = None
            if USE_FP8:
                h0T8 = st.tile([128, NK, Bc], FP8)
                h1T8 = st.tile([128, NK, Bc], FP8)
                nc.vector.memset(h0T8, 0.0)
                nc.vector.memset(h1T8, 0.0)

            # decoder weights (small, persistent)
            spT_sb = dw.tile([128, S], F32)
            nc.sync.dma_start(out=spT_sb, in_=spT_ext[:])
            w0h = dw.tile([128, NK * 128], BF16)
            nc.sync.dma_start(out=w0h, in_=w0h_ext[:])
            w1r = dw.tile([128, 64], BF16)
            nc.sync.dma_start(out=w1r, in_=w1_ext[:])
            woutr = dw.tile([64, 1], BF16)
            nc.sync.dma_start(out=woutr, in_=wout_ext[:])
            ones_r = None
            if use_bl1 or use_b1:
                ones_r = dw.tile([1, 512], BF16)
                nc.vector.memset(ones_r, 1.0)
            bl1r = None
            if use_bl1:
                bl1r = dw.tile([1, G], BF16)
                nc.sync.dma_start(out=bl1r, in_=bl1_ext[:])
            b1r = None
            if use_b1:
                b1r = dw.tile([1, 64], BF16)
                nc.sync.dma_start(out=b1r, in_=b1_ext[:])

            # ================= LSTM phase =================
            with tc.tile_pool(name="lstmw", bufs=1) as lw, \
                 tc.tile_pool(name="gates", bufs=1) as gp, \
                 tc.tile_pool(name="hwork", bufs=1) as hw, \
                 tc.tile_pool(name="lpsum", bufs=1, space="PSUM") as lps:

                xTr = lw.tile([K0, T * Bc], BF16, tag="xTr")
                nc.sync.dma_start(out=xTr, in_=xT_ext[:])
                wx0 = lw.tile([K0, G], BF16, tag="wx0")
                nc.sync.dma_start(out=wx0, in_=wx0_ext[:])
                wh0 = lw.tile([128, NK * G], WDT, tag="wh0")
                nc.sync.dma_start(out=wh0, in_=wh0_ext[:])
                wx1 = lw.tile([128, NK * G], WDT, tag="wx1")
                nc.sync.dma_start(out=wx1, in_=wx1_ext[:])
                wh1 = lw.tile([128, NK * G], WDT, tag="wh1")
                nc.sync.dma_start(out=wh1, in_=wh1_ext[:])

                DR = mybir.MatmulPerfMode.DoubleRow

                def rec_mms(z_ps, w, hT, hT8, starts, stops):
                    """h @ W matmuls: fp8 DoubleRow (2 k-tiles/mm) or bf16."""
                    if USE_FP8:
                        wv = w.rearrange("p (k g) -> p k g", k=NK)
                        for kp in range(NK // 2):
                            for n in range(4):
                                nc.tensor.matmul(
                                    out=z_ps[:, n * 512:(n + 1) * 512],
                                    lhsT=hT8[:, 2 * kp:2 * kp + 2, :],
                                    rhs=wv[:, 2 * kp:2 * kp + 2, n * 512:(n + 1) * 512],
                                    perf_mode=DR,
                                    start=starts and kp == 0,
                                    stop=stops and kp == NK // 2 - 1,
                                )
                    else:
                        for k in range(NK):
                            for n in range(4):
                                nc.tensor.matmul(
                                    out=z_ps[:, n * 512:(n + 1) * 512],
                                    lhsT=hT[:, k, :],
                                    rhs=w[:, k * G + n * 512:k * G + (n + 1) * 512],
                                    start=starts and k == 0,
                                    stop=stops and k == NK - 1,
                                )

                def lstm_mms(z_ps, s, layer):
                    if layer == 0:
                        for n in range(4):
                            nc.tensor.matmul(
                                out=z_ps[:, n * 512:(n + 1) * 512],
                                lhsT=xTr[:, s * Bc:(s + 1) * Bc],
                                rhs=wx0[:, n * 512:(n + 1) * 512],
                                start=True, stop=False,
                            )
                        rec_mms(z_ps, wh0, h0T, h0T8, False, True)
                    else:
                        rec_mms(z_ps, wx1, h0T, h0T8, True, False)
                        rec_mms(z_ps, wh1, h1T, h1T8, False, not use_bl1)
                        if use_bl1:
                            for n in range(4):
                                nc.tensor.matmul(
                                    out=z_ps[:, n * 512:(n + 1) * 512],
                                    lhsT=ones_r[:, 0:Bc],
                                    rhs=bl1r[:, n * 512:(n + 1) * 512],
                                    start=False, stop=True,
                                )

                gsc = {"scale": 1.0 / W_SCALE} if USE_FP8 else {}
                HH = HID // 2  # wavefront half (2 k-tiles)

                def lstm_elem(z_ps, c, hT, hT8, tagp, dma_eng):
                    # z gate columns are host-permuted half-major:
                    # [i0 f0 g0 o0 | i1 f1 g1 o1], 256 wide each.
                    h_bf = hw.tile([Bc, HID], BF16, tag=tagp + "h")
                    for u in (0, 1):
                        sl = slice(u * HH, (u + 1) * HH)
                        zb = u * 1024
                        gif = gp.tile([Bc, 2 * HH], F32, tag=tagp + "gif" + str(u))
                        gg = gp.tile([Bc, HH], F32, tag=tagp + "gg" + str(u))
                        go = gp.tile([Bc, HH], F32, tag=tagp + "go" + str(u))
                        nc.scalar.activation(out=gif, in_=z_ps[:, zb:zb + 512],
                                             func=AF.Sigmoid, **gsc)
                        nc.scalar.activation(out=gg, in_=z_ps[:, zb + 512:zb + 768],
                                             func=AF.Tanh, **gsc)
                        nc.vector.tensor_mul(out=c[:, sl], in0=gif[:, HH:2 * HH], in1=c[:, sl])
                        nc.scalar.activation(out=go, in_=z_ps[:, zb + 768:zb + 1024],
                                             func=AF.Sigmoid, **gsc)
                        nc.vector.tensor_mul(out=gg, in0=gif[:, 0:HH], in1=gg)
                        nc.vector.tensor_add(out=c[:, sl], in0=c[:, sl], in1=gg)
                        tch = hw.tile([Bc, HH], F32, tag=tagp + "tc" + str(u))
                        nc.scalar.activation(out=tch, in_=c[:, sl], func=AF.Tanh)
                        nc.vector.tensor_mul(out=h_bf[:, sl], in0=go, in1=tch)
                        dma_eng.dma_start_transpose(hT[:, 2 * u:2 * u + 2, :], h_bf[:, sl])
                        if USE_FP8:
                            nc.gpsimd.tensor_copy(hT8[:, 2 * u:2 * u + 2, :],
                                                  hT[:, 2 * u:2 * u + 2, :])

                z0_ps = lps.tile([Bc, G], F32, tag="z0")
                z1_ps = lps.tile([Bc, G], F32, tag="z1")
                for s in range(T + 1):
                    if s < T:
                        lstm_mms(z0_ps, s, layer=0)
                    if s >= 1:
                        lstm_mms(z1_ps, s - 1, layer=1)
                    if s < T:
                        lstm_elem(z0_ps, c0, h0T, h0T8, "a", nc.sync)
                    if s >= 1:
                        lstm_elem(z1_ps, c1, h1T, h1T8, "b", nc.sync)

            # ================= decoder phase =================
            with tc.tile_pool(name="dec", bufs=1) as dec, \
                 tc.tile_pool(name="dpsum", bufs=1, space="PSUM") as dps:

                # latent projection lpT[f, b] = sum_h W0[4+h, f] * h1T[h, b]
                lp_ps = dps.tile([128, Bc], F32, tag="zz")
                for k in range(NK):
                    nc.tensor.matmul(
                        out=lp_ps,
                        lhsT=w0h[:, k * 128:(k + 1) * 128],
                        rhs=h1T[:, k, :],
                        start=(k == 0), stop=(k == NK - 1),
                    )
                lpT = dec.tile([128, Bc], F32)
                nc.vector.tensor_copy(lpT, lp_ps)

                # z1T[f, (b, s)] = relu(spT[f, s] + lpT[f, b]) -> bf16
                z1d = dec.tile([128, Bc, S], BF16)
                sp_b = bass.AP(tensor=spT_sb.tensor, offset=spT_sb.offset,
                               ap=[spT_sb.ap[0], [0, Bc], spT_sb.ap[1]])
                lp_b = bass.AP(tensor=lpT.tensor, offset=lpT.offset,
                               ap=[lpT.ap[0], lpT.ap[1], [0, S]])
                nc.vector.tensor_add(out=z1d, in0=sp_b, in1=lp_b)
                z1r = dec.tile([128, Bc * S], BF16)
                nc.scalar.activation(out=z1r, in_=z1d.rearrange("f b s -> f (b s)"),
                                     func=AF.Relu)

                # z2T = relu(W1.T @ z1T + b1); z3 = wout.T @ z2T, gathered
                z3g = dec.tile([NCH * 4, 512], F32)
                for ch in range(NCH):
                    z2_ps = dps.tile([64, 2048], F32, tag="zz")
                    for j in range(4):
                        nc.tensor.matmul(
                            out=z2_ps[:, j * 512:(j + 1) * 512],
                            lhsT=w1r,
                            rhs=z1r[:, ch * 2048 + j * 512:ch * 2048 + (j + 1) * 512],
                            start=True, stop=not use_b1,
                        )
                        if use_b1:
                            nc.tensor.matmul(
                                out=z2_ps[:, j * 512:(j + 1) * 512],
                                lhsT=b1r,
                                rhs=ones_r,
                                start=False, stop=True,
                            )
                    z2r = dec.tile([64, 2048], BF16, tag="z2r")
                    nc.scalar.activation(out=z2r, in_=z2_ps, func=AF.Relu)
                    z3_ps = dps.tile([1, 2048], F32, tag="z3")
                    for j in range(4):
                        nc.tensor.matmul(
                            out=z3_ps[:, j * 512:(j + 1) * 512],
                            lhsT=woutr,
                            rhs=z2r[:, j * 512:(j + 1) * 512],
                            start=True, stop=True,
                        )
                    z3row = dec.tile([1, 2048], F32, tag="z3row")
                    nc.scalar.activation(out=z3row, in_=z3_ps, func=AF.Identity)
                    for j in range(4):
                        nc.sync.dma_start(out=z3g[ch * 4 + j:ch * 4 + j + 1, :],
                                          in_=z3row[:, j * 512:(j + 1) * 512])

                # softplus(x + bout) = relu(x+bout) + ln(exp(-|x+bout|) + 1)
                P16 = NCH * 4
                ax = dec.tile([P16, 512], F32)
                nc.scalar.activation(out=ax, in_=z3g, func=AF.Abs, bias=bout_f)
                ex = dec.tile([P16, 512], F32)
                nc.scalar.activation(out=ex, in_=ax, func=AF.Exp, scale=-1.0)
                ln = dec.tile([P16, 512], F32)
                nc.scalar.activation(out=ln, in_=ex, func=AF.Ln, bias=1.0)
                rl = dec.tile([P16, 512], F32)
                nc.scalar.activation(out=rl, in_=z3g, func=AF.Relu, bias=bout_f)
                spl = dec.tile([P16, 512], F32)
                nc.vector.tensor_add(out=spl, in0=rl, in1=ln)

                # row r = p*512 + q*128 + s2 ; out[b = p*4+q, s2]
                nc.sync.dma_start(
                    out=out_ext[:].rearrange("(p q) s -> p q s", q=4),
                    in_=spl.rearrange("p (q s) -> p q s", q=4),
                )

    nc.compile()
    return nc


_MODULE_CACHE = {}


def _get_module(key, *args):
    if key not in _MODULE_CACHE:
        _MODULE_CACHE[key] = build_module(*args)
    return _MODULE_CACHE[key]


def _prepare(pose_history, sphere_positions, sphere_radii,
             Wx0, Wh0, bl0, Wx1, Wh1, bl1,
             W0, b0, W1, b1, Wout, bout):
    f32 = np.float32
    pose_history = np.asarray(pose_history, f32)
    B, T, D = pose_history.shape
    S = np.asarray(sphere_positions).shape[0]
    Bc = B // N_CORES
    G = 4 * HID

    use_bl0 = bool(np.any(np.asarray(bl0)))
    use_bl1 = bool(np.any(np.asarray(bl1)))
    use_b1 = bool(np.any(np.asarray(b1)))
    bout_f = float(np.asarray(bout, f32).reshape(-1)[0])

    wsc = W_SCALE if USE_FP8 else 1.0
    wdt = F8 if USE_FP8 else BF
    # gate-column permutation to half-major [i0 f0 g0 o0 i1 f1 g1 o1] (256 each)
    HH = HID // 2
    gperm = np.concatenate([np.arange(g * HID + u * HH, g * HID + u * HH + HH)
                            for u in (0, 1) for g in range(4)])
    wx0_h = np.asarray(Wx0, f32)[:, gperm]
    K0 = D + (1 if use_bl0 else 0)
    if use_bl0:
        wx0_h = np.vstack([wx0_h, np.asarray(bl0, f32)[None, gperm]])
    wx0_h = np.ascontiguousarray((wx0_h * wsc).astype(BF))

    def ktile(w, n, dt=BF, sc=1.0):
        return np.ascontiguousarray(
            (np.asarray(w, f32).reshape(NK, 128, n).transpose(1, 0, 2)
             .reshape(128, NK * n) * sc).astype(dt))

    wh0_h = ktile(np.asarray(Wh0, f32)[:, gperm], G, wdt, wsc)
    wx1_h = ktile(np.asarray(Wx1, f32)[:, gperm], G, wdt, wsc)
    wh1_h = ktile(np.asarray(Wh1, f32)[:, gperm], G, wdt, wsc)
    sphere_feat = np.concatenate(
        [np.asarray(sphere_positions, f32), np.asarray(sphere_radii, f32)[:, None]], 1)
    spT_h = np.ascontiguousarray((sphere_feat @ np.asarray(W0, f32)[:4]
                                  + np.asarray(b0, f32)).T)
    w0h_h = ktile(np.asarray(W0, f32)[4:], 128)
    w1_h = np.ascontiguousarray(np.asarray(W1, f32).astype(BF))
    wout_h = np.ascontiguousarray(np.asarray(Wout, f32).astype(BF))

    nc = _get_module((T, Bc, K0, S, use_bl1, use_b1, bout_f),
                     T, Bc, K0, S, use_bl1, use_b1, bout_f)

    in_maps = []
    for c in range(N_CORES):
        pc = pose_history[c * Bc:(c + 1) * Bc]  # [Bc, T, D]
        xT = pc.transpose(2, 1, 0).reshape(D, T * Bc)
        if use_bl0:
            xT = np.vstack([xT, np.ones((1, T * Bc), f32)])
        m = {
            "xT": np.ascontiguousarray(xT.astype(BF)), "wx0": wx0_h, "wh0": wh0_h,
            "wx1": wx1_h, "wh1": wh1_h, "spT": spT_h, "w0h": w0h_h,
            "w1": w1_h, "wout": wout_h,
        }
        if use_bl1:
            m["bl1"] = np.ascontiguousarray(
                (np.asarray(bl1, f32).reshape(1, G) * wsc).astype(BF))
        if use_b1:
            m["b1"] = np.ascontiguousarray(np.asarray(b1, f32).reshape(1, 64).astype(BF))
        in_maps.append(m)

    return nc, in_maps


def kernel(pose_history, sphere_positions, sphere_radii,
           Wx0, Wh0, bl0, Wx1, Wh1, bl1,
           W0, b0, W1, b1, Wout, bout):
    nc, in_maps = _prepare(pose_history, sphere_positions, sphere_radii,
                           Wx0, Wh0, bl0, Wx1, Wh1, bl1,
                           W0, b0, W1, b1, Wout, bout)
    res = run_bass_kernel_spmd(nc, in_maps, list(range(N_CORES)))
    out = np.concatenate([res.results[c]["out"] for c in range(N_CORES)], axis=0)
    return out.astype(np.float32)


def time_kernel(reps=20, **inputs):
    """Min wall-clock of repeated device executions with device-resident inputs."""
    import time

    import jax
    from jax.experimental.shard_map import shard_map
    from jax.sharding import Mesh, NamedSharding, PartitionSpec

    from concourse import bass2jax, mybir as _mybir

    nc, in_maps = _prepare(**inputs)
    bass2jax.install_neuronx_cc_hook()

    part_name = nc.partition_id_tensor.name if nc.partition_id_tensor else None
    in_names, out_names, out_avals, zero_outs = [], [], [], []
    for alloc in nc.m.functions[0].allocations:
        if not isinstance(alloc, _mybir.MemoryLocationSet):
            continue
        name = alloc.memorylocations[0].name
        if alloc.kind == "ExternalInput":
            if name != part_name:
                in_names.append(name)
        elif alloc.kind == "ExternalOutput":
            shape = tuple(alloc.tensor_shape)
            dtype = _mybir.dt.np(alloc.dtype)
            out_names.append(name)
            out_avals.append(jax.core.ShapedArray(shape, dtype))
            zero_outs.append(np.zeros(shape, dtype))
    n_params = len(in_names)
    all_names = in_names + out_names
    if part_name is not None:
        all_names = all_names + [part_name]

    def _body(*args):
        operands = list(args)
        if part_name is not None:
            operands.append(bass2jax.partition_id_tensor())
        outs = bass2jax._bass_exec_p.bind(
            *operands,
            out_avals=tuple(out_avals),
            in_names=tuple(all_names),
            out_names=tuple(out_names),
            lowering_input_output_aliases=(),
            sim_require_finite=True,
            sim_require_nnan=True,
            nc=nc,
        )
        return tuple(outs)

    devices = jax.devices()[:N_CORES]
    mesh = Mesh(np.asarray(devices), ("core",))
    nin = n_params + len(zero_outs)
    sharded = jax.jit(
        shard_map(_body, mesh=mesh,
                  in_specs=(PartitionSpec("core"),) * nin,
                  out_specs=(PartitionSpec("core"),) * len(out_names),
                  check_rep=False),
        keep_unused=True,
    )
    sh = NamedSharding(mesh, PartitionSpec("core"))
    dev_in = [
        jax.device_put(np.concatenate([in_maps[c][n] for c in range(N_CORES)], 0), sh)
        for n in in_names
    ] + [
        jax.device_put(np.concatenate([z] * N_CORES, 0), sh) for z in zero_outs
    ]
    # warmup (compiles via NEFF cache)
    jax.block_until_ready(sharded(*dev_in))
    best = float("inf")
    for _ in range(reps):
        t0 = time.perf_counter()
        jax.block_until_ready(sharded(*dev_in))
        best = min(best, time.perf_counter() - t0)
    return best * 1e9
